# revision 5
# baseline (speedup 1.0000x reference)
"""AWQ 4-bit quantized linear layer on 8 Trainium2 NeuronCores.

Problem: out = x @ dequant(qweight, scales, qzeros) + bias
  x       [8192, 4096] fp16   (replicated to all cores, pre-transposed on host)
  qweight [4096, 1536] int32  (8x int4 nibbles packed along out_features)
  scales  [32, 12288]  fp16   (group_size=128 along in_features)
  qzeros  [32, 1536]   int32  (packed like qweight)
  bias    [12288]      fp16
  out     [8192, 12288] fp16

Sharding: tensor-parallel colwise. out_features 12288 -> 8 shards of 1536.
Each core computes out[:, shard] independently; host concatenates. x is
replicated, transposed on host so the contraction dim lands on SBUF
partitions with plain DMAs.

v2 (from the 1.441 ms baseline, bf16 matmul floor is ~1.327 ms = 6144
matmuls at the 216 ns N=512 issue-rate floor; the loss was concentrated in
the dequant startup phase: first MM at 31.8 us, ~44 us of PE gaps and
~30 us of HAM cold penalty while DVE dequantized W):
  1. Group metadata [s | z*s] is precomputed on the host (0.4% of the
     bytes; W itself is still dequantized on device) and passed as a DRAM
     input, so per-k-tile rows broadcast straight from DRAM - no device
     zero-point unpack, no DRAM scratch roundtrip. First w-tile is ready
     ~6 us in.
  2. qweight is viewed as int16 on the host: nibble unpack is 4 DVE ops
     (16-bit, 2x-eligible) instead of 8 int32 ops, batched 2 k-tiles per
     op. Columns use a per-core PERMUTED order (position jj*384 + 2c + h
     holds feature 8c + 4h + jj) so every unpack op writes one contiguous
     block; scales/zs/bias are permuted and the output unpermuted on host.
  3. The first m-superchunk is processed k-PHASED: 8 psum accumulators
     (4 m-tiles x o-tiles 0,1) stay open while k-tiles stream in, so the
     PE consumes each w-tile the moment dequant produces it (~1.7 us/tile
     consumption vs ~2.5 us/tile production) instead of stalling 60 us for
     the full W. Phase B (o=2) then runs at full rate. HAM stays warm.
  4. Output DMA per [128,512] o-slice right after its bias-add.
"""

import sys

for p in ("/opt/trn_rl_repo", "/opt/pypackages"):
    if p not in sys.path:
        sys.path.insert(0, p)

import numpy as np

import concourse.bacc as bacc
import concourse.bass as bass
import concourse.mybir as mybir
from concourse.tile import TileContext

f16 = mybir.dt.float16
f32 = mybir.dt.float32
i16 = mybir.dt.int16
Alu = mybir.AluOpType

N_CORES = 8
M_FULL, K_FULL, O_FULL = 8192, 4096, 12288
GROUP_SIZE = 128
PACK = 8  # int4 values per int32

O_SHARD = O_FULL // N_CORES        # 1536
C_SHARD = O_SHARD // PACK          # 192 int32 columns = 384 int16 columns


def _perm(C):
    """Per-core column permutation for the int16 nibble unpack: permuted
    position jj*(2C) + 2c + h holds the natural out-feature 8c + 4h + jj
    (c: int32 word, h: int16 half, jj: nibble within the int16). Each of
    the 4 unpack ops then writes one contiguous 2C-wide block. scales/zs/
    bias are permuted on the host; the output is unpermuted on the host."""
    p2f = np.empty(PACK * C, dtype=np.int64)
    for jj in range(4):
        for c in range(C):
            for h in range(2):
                p2f[jj * 2 * C + 2 * c + h] = 8 * c + 4 * h + jj
    return p2f


def build_nc(M=M_FULL, K=K_FULL, O=O_SHARD, MS=512, xt_bufs=52):
    """Build the per-core Bass program (SPMD: same program on all cores)."""
    KT = K // 128                  # k-tiles == quant groups per shard (32)
    G = K // GROUP_SIZE
    assert KT == G, "kernel assumes group_size == 128 == k-tile"
    C2 = O // 4                    # int16 columns per shard (384)
    OT = O // 512                  # o-tiles of 512 (3)
    NMS = M // MS                  # m-superchunks (16)
    MT = MS // 128                 # m-tiles per superchunk (4)
    NP = KT // 2                   # k-tile pairs (16)

    nc = bacc.Bacc("TRN2")
    xt_in = nc.dram_tensor("xt", [K, M], f16, kind="ExternalInput")
    qw16 = nc.dram_tensor("qw16", [K, C2], i16, kind="ExternalInput")
    # host-precomputed group metadata rows: [:, :O] = s, [:, O:] = z*s
    # (permuted column order)
    ssz = nc.dram_tensor("ssz", [G, 2 * O], f16, kind="ExternalInput")
    bias = nc.dram_tensor("bias", [1, O], f16, kind="ExternalInput")
    out = nc.dram_tensor("out", [M, O], f16, kind="ExternalOutput")

    with TileContext(nc) as tc:
        with (
            tc.tile_pool(name="wres", bufs=NP) as w_pool,
            tc.tile_pool(name="xt", bufs=xt_bufs) as xt_pool,
            tc.tile_pool(name="qwc", bufs=2) as qwc_pool,
            tc.tile_pool(name="stage", bufs=2) as stage_pool,
            tc.tile_pool(name="bc", bufs=3) as bc_pool,
            tc.tile_pool(name="meta", bufs=1) as meta_pool,
            tc.tile_pool(name="obuf", bufs=4) as o_pool,
            tc.tile_pool(name="psum", bufs=8, space="PSUM") as psum_pool,
        ):
            # ---- superchunk-0 x tiles start immediately on the SP ring ----
            xts0 = []
            for t in range(KT):
                xt = xt_pool.tile([128, MS], f16, tag="xt", name="xt")
                nc.sync.dma_start(xt[:], xt_in[t * 128:(t + 1) * 128, 0:MS])
                xts0.append(xt)

            # ---- bias broadcast [128, O] (ACT ring) ----
            bias_b = meta_pool.tile([128, O], f16, tag="biasb")
            nc.scalar.dma_start(bias_b[:], bias[0, :].partition_broadcast(128))

            # ---- dequantize W shard into resident SBUF, 2 k-tiles/batch ----
            # qw rows are k = t*128 + p; pair u covers k-tiles 2u, 2u+1.
            qw_r = qw16.rearrange("(t p) c -> p t c", p=128)
            w_tiles = []   # w_tiles[t] view [128, O]
            for u in range(NP):
                qw_c = qwc_pool.tile([128, 2, C2], i16, tag="qwc", name="qwc")
                nc.sync.dma_start(qw_c[:], qw_r[:, 2 * u:2 * u + 2, :])
                # int16 nibble unpack: 4 ops, each writes a contiguous block
                wq_i = stage_pool.tile([128, 2, O], i16, tag="wqi", name="wqi")
                for jj in range(4):
                    nc.vector.tensor_scalar(
                        wq_i[:, :, jj * C2:(jj + 1) * C2],
                        qw_c[:], 4 * jj, 0xF,
                        Alu.logical_shift_right, Alu.bitwise_and,
                    )
                w2 = w_pool.tile([128, 2, O], f16, tag="w", name="w")
                for v in range(2):
                    t = 2 * u + v
                    # [s | zs] row broadcast; halves split across DMA rings
                    ssz_b = bc_pool.tile([128, 2 * O], f16, tag="sszb",
                                         name="sszb")
                    nc.scalar.dma_start(
                        ssz_b[:, :O], ssz[t, :O].partition_broadcast(128))
                    nc.sync.dma_start(
                        ssz_b[:, O:], ssz[t, O:].partition_broadcast(128))
                    # int16 -> f16 cast on ACT (own SBUF port)
                    wq_f = bc_pool.tile([128, O], f16, tag="wqf", name="wqf",
                                        bufs=2)
                    nc.scalar.copy(wq_f[:], wq_i[:, v, :])
                    # w = wq * s - zs  (f16 2x mode on DVE)
                    nc.vector.tensor_tensor(
                        w2[:, v, :], wq_f[:], ssz_b[:, :O], Alu.mult)
                    nc.vector.tensor_tensor(
                        w2[:, v, :], w2[:, v, :], ssz_b[:, O:], Alu.subtract)
                    w_tiles.append(w2[:, v, :])

            def evict(ps, mi, o, ms_base):
                """psum -> sbuf (ACT), bias add (DVE), DMA out (SP ring)."""
                ob = o_pool.tile([128, 512], f16, tag="ob", name="ob")
                nc.scalar.copy(ob[:], ps[:])
                nc.vector.tensor_tensor(
                    ob[:], ob[:], bias_b[:, o * 512:(o + 1) * 512], Alu.add)
                m0 = ms_base + mi * 128
                nc.sync.dma_start(
                    out[m0:m0 + 128, o * 512:(o + 1) * 512], ob[:])

            # ---- superchunk 0: k-phased phase A (o=0,1), then phase B ----
            # Phase A: 8 open accumulators consume each w-tile as dequant
            # produces it; accumulation groups interleave across banks.
            psA = []
            for mi in range(MT):
                for o in range(2):
                    ps = psum_pool.tile([128, 512], f32, tag="ps",
                                        name="ps")
                    psA.append((mi, o, ps))
            for t in range(KT):
                for mi, o, ps in psA:
                    nc.tensor.matmul(
                        ps[:],
                        xts0[t][:, mi * 128:(mi + 1) * 128],
                        w_tiles[t][:, o * 512:(o + 1) * 512],
                        start=(t == 0), stop=(t == KT - 1),
                        skip_group_check=True,
                    )
            for mi, o, ps in psA:
                evict(ps, mi, o, 0)
            # Phase B: o=2 at full rate (w resident now)
            for mi in range(MT):
                ps = psum_pool.tile([128, 512], f32, tag="ps", name="ps")
                for t in range(KT):
                    nc.tensor.matmul(
                        ps[:],
                        xts0[t][:, mi * 128:(mi + 1) * 128],
                        w_tiles[t][:, 2 * 512:3 * 512],
                        start=(t == 0), stop=(t == KT - 1),
                    )
                evict(ps, mi, 2, 0)

            # ---- superchunks 1..NMS-1: stream xT, accumulate, evict ----
            for ms in range(1, NMS):
                xts = []
                for t in range(KT):
                    xt = xt_pool.tile([128, MS], f16, tag="xt", name="xt")
                    nc.sync.dma_start(
                        xt[:],
                        xt_in[t * 128:(t + 1) * 128, ms * MS:(ms + 1) * MS],
                    )
                    xts.append(xt)
                for mi in range(MT):
                    for o in range(OT):
                        ps = psum_pool.tile([128, 512], f32, tag="ps",
                                            name="ps")
                        for t in range(KT):
                            nc.tensor.matmul(
                                ps[:],
                                xts[t][:, mi * 128:(mi + 1) * 128],
                                w_tiles[t][:, o * 512:(o + 1) * 512],
                                start=(t == 0), stop=(t == KT - 1),
                            )
                        evict(ps, mi, o, ms * MS)

    if not nc.is_finalized():
        nc.finalize()
    return nc


def _unpack_int4_np(q):
    """[R, Cpacked] int32 -> [R, Cpacked*8] int4 values (nibble j -> col c*8+j)."""
    shifts = (np.arange(PACK, dtype=np.int32) * 4)[None, None, :]
    return ((q[:, :, None] >> shifts) & 0xF).reshape(q.shape[0], -1)


def _shard_inputs(x, qweight, scales, qzeros, bias):
    xt_full = np.ascontiguousarray(np.asarray(x).T)  # [K, M], replicated
    perm = _perm(C_SHARD)
    zq_full = _unpack_int4_np(np.asarray(qzeros))    # [G, O_FULL] int
    scales = np.asarray(scales)
    in_maps = []
    for c in range(N_CORES):
        so = slice(c * O_SHARD, (c + 1) * O_SHARD)
        sc = slice(c * C_SHARD, (c + 1) * C_SHARD)
        s_p = scales[:, so][:, perm].astype(np.float32)
        zs_p = zq_full[:, so][:, perm].astype(np.float32) * s_p
        ssz = np.concatenate([s_p, zs_p], axis=1).astype(np.float16)
        qw16 = np.ascontiguousarray(
            np.asarray(qweight)[:, sc]).view(np.int16)
        in_maps.append({
            "xt": xt_full,
            "qw16": qw16,
            "ssz": np.ascontiguousarray(ssz),
            "bias": np.ascontiguousarray(
                np.asarray(bias)[so][perm]).reshape(1, -1),
        })
    return in_maps


_CACHED_NC = None


def kernel(x, qweight, scales, qzeros, bias):
    from concourse.bass_utils import run_bass_kernel_spmd

    global _CACHED_NC
    if _CACHED_NC is None:
        _CACHED_NC = build_nc()
    nc = _CACHED_NC

    in_maps = _shard_inputs(x, qweight, scales, qzeros, bias)
    res = run_bass_kernel_spmd(nc, in_maps, core_ids=list(range(N_CORES)))
    # undo the per-core column permutation while gathering
    perm = _perm(C_SHARD)
    out = np.empty((M_FULL, O_FULL), dtype=np.float16)
    for c in range(N_CORES):
        out[:, c * O_SHARD + perm] = res.results[c]["out"]
    return out
